# revision 56
# baseline (speedup 1.0000x reference)
"""Trainium2 Bass kernel for nn_CompetitiveNetwork (competitive-binding solve).

Final design, ~3.9x faster than the 21-plain-iteration baseline:
  - Steffensen acceleration: two blocks of (4, 3) fixed-point iterations,
    each followed by a per-element Aitken extrapolation. 7 matmul-pair
    iterations replace the reference's 21 (contraction ~0.6/iter).
  - All reciprocals are SEEDLESS: bf on ACT (Reciprocal, +1 via bias);
    af = AT/(1+T) in ONE custom DVE op (DIV1P: bitwise-NOT seed + one
    Newton pass + multiply), so no Pool muls and no Newton-seed hazards.
  - Aitken: d2/den built on the idle PE via +/-identity matmuls into
    PSUM; corr = d2*clip(d2*rden, +-4) (CORRCLAMP custom op) guards the
    fp16 quantization blow-up; af' = h2-corr is never materialized --
    consumers fold the subtraction via -w1/-m2 stationaries.
  - 2 packed input DMAs, parallel per-stream output copy+DMA tail.


Math per batch row b (host precomputes K = clip(exp(K_raw)), BT = 1,
w1 = K, w2 = (K*BT)^T, m2 = K*clip(W)*BT):
    iterate:  bf = 1/(1 + K^T af);  af = AT * 1/(1 + w2^T bf)
    readout:  y  = sum_j (m2^T af)_j * bf_j + b

Acceleration: Steffensen — two blocks of (4, 3) fixed-point iterations,
each followed by a per-element Aitken extrapolation (x2 - d2^2/(d2-d1)).
7 iterations + 2 extrapolations match the 21-iteration reference to
~2-3e-3 relative (fixed-point contraction ~0.6/iter; validated vs the
fp32 reference in numpy with exact fp16/fp32 device arithmetic).

Device layout: data-parallel over batch (16384 -> 8 cores x 2048);
transposed state (64 features on partitions, batch on free), two
64-partition streams stacked to (128, FD) tiles, 4 chunks of FD=256.
Matmuls use 128x128 block-diagonal stationaries (one instruction per
chunk instead of two 64x64 quadrant matmuls).

Engine plan per iteration (layered emission for cross-chunk overlap):
    PE : mmS[c] (block-diag w1, fp16) ; bias row + mmT[c] accumulating
         1 + w2^T bf into PSUM
    ACT: bf[c] = Reciprocal(S[c] + 1) -> fp16   (bias folds the +1)
    DVE: r[c] = reciprocal_approx_fast(1+T[c]) -> fp32 (seedless, ~18 bits)
    Pool: af[c] = att16[c] * r[c] -> fp16
All reciprocals are seedless/exact: no Newton-seed divergence hazards.
"""

import numpy as np

import concourse.bacc as bacc
import concourse.mybir as mybir
from concourse.tile import TileContext
from concourse.bass_utils import run_bass_kernel_spmd

# --- custom DVE op CORRCLAMP_ANT: out = Src0 * clip(Src0*Src1, C0, C1) ---
# The Aitken correction d2^2/den with the ratio d2*(1/den) clamped to the
# range valid for geometric sequences (rate <= 0.65 => |d2/den| <= ~3).
# Clamping kills the blow-up when consecutive fp16 differences quantize
# equal (den ~ 0 while d2 is a 1-2 ulp residue).

import concourse.dve_ops as dve_ops
from concourse.dve_ops import DveOp
from concourse.dve_spec import (Spec, Src0, Src1, C0, C1, C2, lower, maxx,
                                minn, Bin, AluOp)


def _ref_corrclamp(in0, in1, c0, c1, c2):
    q = np.clip(in0.astype(np.float32) * in1.astype(np.float32), c0, c1)
    return (q * in0.astype(np.float32)).astype(np.float32)


_not_x = Bin(AluOp.BITWISE_NOT, Src0, Src0)
_y0 = _not_x * C0
_y1 = _y0 * (C1 - Src0 * _y0)

# bit-flip seed + one NR pass + multiply: out = Src1 * ~1/Src0, rel err
# ~1.7e-3 (one Newton pass short of RECIPROCAL_APPROX_FAST). Good enough
# inside the fixed-point loop where the contraction damps recip noise.
def _ref_div1p(in0, in1, c0, c1, c2):
    x = in0.astype(np.float32)
    not_x = (~x.view(np.int32)).view(np.float32)
    y0 = not_x * np.float32(c0)
    y1 = y0 * (np.float32(c1) - x * y0)
    return (y1 * in1.astype(np.float32)).astype(np.float32)


# DIV1P2: same seedless divide but with the +1 folded into the op
# (x1 = Src0 + 1), so the PE bias-row matmul into sT is unnecessary.
from concourse.dve_spec import One as _One

_x1 = Src0 + _One
_not_x1 = Bin(AluOp.BITWISE_NOT, _x1, _x1)
_y0b = _not_x1 * C0
_y1b = _y0b * (C1 - _x1 * _y0b)


def _ref_div1p2(in0, in1, c0, c1, c2):
    x = np.ascontiguousarray(in0.astype(np.float32) + np.float32(1.0))
    not_x = (~x.view(np.int32)).view(np.float32)
    y0 = not_x * np.float32(c0)
    y1 = y0 * (np.float32(c1) - x * y0)
    return (y1 * in1.astype(np.float32)).astype(np.float32)


# CORRSMOOTH: corr = k*d2^2*den' / (d2^2 + den'^2), where the CALLER
# supplies den' = k*den + bias accumulated on the PE (scaled +-kI identity
# stationaries plus a 2^-12 bias row). This is the smooth equivalent of
# clip(d2/den, +-k/2)*d2 with graceful 0/0 -> 0 handling: no clamp, no NaN
# path, no ACT rden at all. The DVE pipeline budget is 8 ALU ops, so the
# reciprocal is SEED-ONLY (bitwise-NOT, rel err ~6%, benign on a damped
# extrapolation correction) and the k fold rides inside the seed constant:
# ops = [d2^2, den'^2, +, NOT, *C2, d2*den', *d2, *y0] = exactly 8.
# Src0 = d2, Src1 = den'; C2 = k * seed_scale (C0/C1 unused).

_x = Src0 * Src0 + Src1 * Src1
_nx = Bin(AluOp.BITWISE_NOT, _x, _x)
_ry0 = _nx * C2


def _ref_corrsmooth(in0, in1, c0, c1, c2):
    d2 = in0.astype(np.float32)
    den = in1.astype(np.float32)
    x = np.ascontiguousarray(d2 * d2 + den * den)
    not_x = (~x.view(np.int32)).view(np.float32)
    y0 = not_x * np.float32(c2)
    return (((d2 * den) * d2) * y0).astype(np.float32)


_SPECS = {
    "CORRCLAMP_ANT": Spec(body=minn(maxx(Src0 * Src1, C0), C1) * Src0,
                          reference=_ref_corrclamp),
    "DIV1P_ANT": Spec(body=_y1 * Src1, reference=_ref_div1p),
    "DIV1P2_ANT": Spec(body=_y1b * Src1, reference=_ref_div1p2),
    "CORRSMOOTH_ANT": Spec(body=((Src0 * Src1) * Src0) * _ry0,
                           reference=_ref_corrsmooth),
}


def _make_op(name, shas):
    return DveOp(name, _SPECS[name], subdim=False, uops_sha=shas)


def _register(name):
    for op in dve_ops.OPS:
        if op.name == name:
            return op
    probe = _make_op(name, {})
    shas = {}
    for ver in ("v3", "v4"):
        try:
            from concourse.dve_uop import DveOpSpec
            opcode = dve_ops._CUSTOM_DVE_ROW_BASE + len(dve_ops.OPS)
            res = DveOpSpec(name=probe.name, opcode=opcode,
                            uops=lower(probe.spec, ver=ver),
                            rd1_en=True)
            shas[ver] = res.sha(ver)
        except Exception as e:
            print(f"lower {name} {ver} failed: {e}")
    op = _make_op(name, shas)
    dve_ops.OPS.append(op)
    dve_ops.CUSTOM_DVE_SPECS[op.name] = op.spec
    dve_ops._SUB_OPCODE_FOR_NAME[op.name] = (
        dve_ops._CUSTOM_DVE_ROW_BASE + len(dve_ops.OPS) - 1)
    return op


def corrclamp(nc_vector, out, d2, rden, lim=4.0):
    op = _register("CORRCLAMP_ANT")
    return nc_vector._custom_dve(op, out=out, in0=d2, in1=rden,
                                 s0=-lim, s1=lim, imm2=0.0)


def div1p(nc_vector, out, x, num):
    op = _register("DIV1P_ANT")
    return nc_vector._custom_dve(op, out=out, in0=x, in1=num,
                                 s0=-0.23549792, s1=2.0017324, imm2=0.0)


def div1p2(nc_vector, out, x, num):
    """out = num / (1 + x), seedless (folds the +1; no bias row needed)."""
    op = _register("DIV1P2_ANT")
    return nc_vector._custom_dve(op, out=out, in0=x, in1=num,
                                 s0=-0.23549792, s1=2.0017324, imm2=0.0)


AIT_K = 32.0                   # den scale; cap |d2/den| at k/2
AIT_BIAS = 2.0 ** -12          # den' bias row value (zero-protection)
_SEED_A = -0.23528764          # seed-only NOT-reciprocal scale


def corrsmooth(nc_vector, out, d2, den):
    op = _register("CORRSMOOTH_ANT")
    return nc_vector._custom_dve(op, out=out, in0=d2, in1=den,
                                 s0=0.0, s1=0.0, imm2=AIT_K * _SEED_A)

B, NA, NB = 16384, 64, 64
N_CORES = 8
B_CORE = B // N_CORES          # 2048 batch rows per core
N_CHUNK = 4
FD = B_CORE // 2 // N_CHUNK    # 256
N1, N2 = 3, 3                  # Steffensen blocks

FP32 = mybir.dt.float32
FP16 = mybir.dt.float16

_CACHE = {}


def _act_recip(nc, out_ap, in_ap, bias=1.0):
    eng = nc.scalar
    ins = [eng.lower_ap(in_ap),
           mybir.ImmediateValue(dtype=FP32, value=bias),   # bias: +1
           mybir.ImmediateValue(dtype=FP32, value=1.0),    # scale
           mybir.ImmediateValue(dtype=FP32, value=0.0)]
    eng.add_instruction(mybir.InstActivation(
        name=nc.get_next_instruction_name(),
        func=mybir.ActivationFunctionType.Reciprocal,
        ins=ins, outs=[eng.lower_ap(out_ap)]))


def _build_module(repeat=1, n1=N1, n2=N2, n3=None, do_aitken=True,
                  do_readout=True,
                  mul_eng="pool", div1p_mode="all", ait_eng="pe",
                  bf_eng=("act", "act", "act", "act"),
                  af_eng=("dve", "dve", "dve", "dve"),
                  ro_bf_eng=("act", "act", "act", "act"),
                  ro_h_eng=("dve", "dve", "dve", "dve"),
                  tail_mode="copy"):
    nc = bacc.Bacc()
    # packA: w1 | w2 | att0  (first-iteration critical data: arrives first)
    # packB: att1 | att2 | att3
    # pack2: m2 | I | -I | -2I | -w1 | -m2 (readout + aitken consts)
    packA = nc.dram_tensor("packA", (128, 256 + 2 * FD), FP16,
                           kind="ExternalInput")
    packB = nc.dram_tensor("packB", (128, (N_CHUNK - 2) * FD), FP16,
                           kind="ExternalInput")
    pack2 = nc.dram_tensor("pack2", (128, 768), FP16, kind="ExternalInput")
    # rows 0:8 carry the result; rows 8:16 are scatter-mode padding (the
    # SWDGE scatter needs 16 non-negative indices to write rows 0-7 cleanly)
    yout = nc.dram_tensor("yout", (16, FD), FP32, kind="ExternalOutput")
    if tail_mode == "scatter":
        idxs_d = nc.dram_tensor("idxs", (16, 1), mybir.dt.int16,
                                kind="ExternalInput")

    with TileContext(nc) as tc, \
         tc.tile_pool(name="const", bufs=1) as cpool, \
         tc.tile_pool(name="state", bufs=5) as spool, \
         tc.tile_pool(name="work", bufs=8) as wpool, \
         tc.tile_pool(name="psA", bufs=1, space="PSUM") as ppA, \
         tc.tile_pool(name="psB", bufs=1, space="PSUM") as ppB:

        p1a = cpool.tile([128, 256 + 2 * FD], FP16, tag="p1a")
        p1b = cpool.tile([128, (N_CHUNK - 2) * FD], FP16, tag="p1b")
        p2 = cpool.tile([128, 768], FP16, tag="p2")
        nc.sync.dma_start(out=p1a[:], in_=packA[:, :])
        nc.sync.dma_start(out=p1b[:], in_=packB[:, :])
        nc.sync.dma_start(out=p2[:], in_=pack2[:, :])
        w1f = p1a[:, 0:128]
        w2f = p1a[:, 128:256]
        atts = [p1a[:, 256 + c * FD:256 + (c + 1) * FD] for c in range(2)] + \
               [p1b[:, c * FD:(c + 1) * FD] for c in range(N_CHUNK - 2)]
        m2f = p2[:, 0:128]
        idf = p2[:, 128:256]      # I
        nidf = p2[:, 256:384]     # -I
        n2idf = p2[:, 384:512]    # -2I
        nw1f = p2[:, 512:640]     # -w1
        nm2f = p2[:, 640:768]     # -m2
        one_l = cpool.tile([1, 128], FP16, tag="one_l")   # bias lhsT (1,128)
        nc.vector.memset(one_l[:], 1.0)
        one_r = cpool.tile([1, FD], FP16, tag="one_r")    # bias rhs (1,FD)
        nc.vector.memset(one_r[:], 1.0)
        tiny_r = cpool.tile([1, FD], FP16, tag="tiny_r")  # den' bias row
        nc.vector.memset(tiny_r[:], AIT_BIAS)
        ones_t = cpool.tile([128, FD], FP16, tag="ones_t")  # Pool bf numerator
        nc.vector.memset(ones_t[:], 1.0)
        # readout reduction mask (merged streams): chunk c uses columns
        # [8c:8c+8]; col 8c+j has ones on partitions 0:64 iff j==c (stream A
        # sums land in out rows 0:4), col 8c+4+j ones on partitions 64:128
        # iff j==c (stream B sums land in out rows 4:8).
        ymask = cpool.tile([128, 8 * N_CHUNK], FP16, tag="ymask")
        nc.vector.memset(ymask[:], 0.0)
        for c in range(N_CHUNK):
            nc.vector.memset(ymask[0:64, 8 * c + c:8 * c + c + 1], 1.0)
            nc.vector.memset(ymask[64:128, 8 * c + 4 + c:8 * c + 4 + c + 1], 1.0)

        if tail_mode == "scatter":
            # y leaves via a pre-generated SWDGE scatter-add: descriptors are
            # built off the critical path, so the tail is just copy ->
            # trigger -> tiny transfer instead of a full dma_start chain
            # (config + HWDGE issuance + DGE delay). yout is zeroed early by
            # an overlapped DMA since the scatter ADDs into it.
            from concourse import library_config
            nc.gpsimd.load_library(library_config.mlp)
            idxs_t = cpool.tile([16, 1], mybir.dt.int16, tag="idxs")
            nc.sync.dma_start(out=idxs_t[:], in_=idxs_d[:, :])
            zeros_t = cpool.tile([128, FD], FP32, tag="zeros")
            nc.vector.memset(zeros_t[:], 0.0)
            nc.sync.dma_start(out=yout[:, :], in_=zeros_t[0:16, :])
            ys3 = cpool.tile([128, 1, FD], FP32, tag="ys3")
            nc.vector.memset(ys3[:, :, :], 0.0)
            ydma_sem = nc.alloc_semaphore("ydma")
            nc.gpsimd.dma_scatter_add(yout[:, :], ys3[:, :, :], idxs_t[:],
                                      16, 16, FD, prepare_only=True,
                                      sem=ydma_sem)
        else:
            ys3 = ydma_sem = None

        for _rep in range(repeat):
            af = [None] * N_CHUNK

            def iteration(ait=None):
                """One fixed-point iteration over all chunks, layered.

                Per-chunk engine lanes (bf_eng / af_eng): "act" = ACT
                Reciprocal (bias folds the +1), "dve" = DIV1P2 custom op
                (folds the +1), "pool" = GPSIMD exact divide (needs an
                explicit +1 bias row accumulated into its PSUM tile).
                """
                sA, sT, bfs = [], [], []
                # two passes for extrapolated inputs: the w1*h2 halves have
                # no corr dependency, so emit them all first — the PE works
                # through them while the corr chain (den->rden->corrclamp)
                # is still in flight, instead of head-of-line blocking.
                for c in range(N_CHUNK):
                    ps_t = ppA.tile([128, FD], FP32, tag=f"sA{c}", name=f"sA{c}")
                    ps = ps_t[:]
                    need_bias = bf_eng[c] == "pool"
                    if af[c] is None:
                        nc.tensor.matmul(out=ps, lhsT=w1f, rhs=atts[c],
                                         start=True, stop=not need_bias)
                    elif isinstance(af[c], tuple):
                        nc.tensor.matmul(out=ps, lhsT=w1f, rhs=af[c][0][:],
                                         start=True, stop=False)
                    else:
                        nc.tensor.matmul(out=ps, lhsT=w1f, rhs=af[c][:],
                                         start=True, stop=not need_bias)
                    sA.append(ps)
                for c in range(N_CHUNK):
                    need_bias = bf_eng[c] == "pool"
                    if isinstance(af[c], tuple):
                        nc.tensor.matmul(out=sA[c], lhsT=nw1f, rhs=af[c][1][:],
                                         start=False, stop=not need_bias)
                    if need_bias:
                        nc.tensor.matmul(out=sA[c], lhsT=one_l[:], rhs=one_r[:],
                                         start=False, stop=True)
                # sT bias rows only for pool-af chunks (ACT folds via bias,
                # DVE folds via DIV1P2)
                for c in range(N_CHUNK):
                    ps2_t = ppB.tile([128, FD], FP32, tag=f"sT{c}", name=f"sT{c}")
                    sT.append(ps2_t[:])
                    if af_eng[c] == "pool":
                        nc.tensor.matmul(out=sT[c], lhsT=one_l[:], rhs=one_r[:],
                                         start=True, stop=False)
                def emit_bf(c):
                    bf = wpool.tile([128, FD], FP16, tag=f"bf{c}")
                    if bf_eng[c] == "act":
                        _act_recip(nc, bf[:], sA[c])
                    else:
                        nc.gpsimd.tensor_tensor(bf[:], ones_t[:], sA[c],
                                                op=mybir.AluOpType.divide)
                    bfs.append(bf)

                def emit_mmT(c):
                    first = af_eng[c] != "pool"
                    nc.tensor.matmul(out=sT[c], lhsT=w2f, rhs=bfs[c][:],
                                     start=first, stop=True)

                def emit_af(c):
                    af_n = spool.tile([128, FD], FP16, tag=f"af{c}")
                    if af_eng[c] == "dve":
                        div1p2(nc.vector, af_n[:], sT[c], atts[c])
                    else:
                        nc.gpsimd.tensor_tensor(af_n[:], atts[c], sT[c],
                                                op=mybir.AluOpType.divide)
                    af[c] = af_n

                for c in range(N_CHUNK):
                    emit_bf(c)
                if ait is None:
                    for c in range(N_CHUNK):
                        emit_mmT(c)
                    for c in range(N_CHUNK):
                        emit_af(c)
                    return None
                # Last iteration of a Steffensen block: interleave the
                # aitken den/d2 identity-matmuls with this iteration's mmT
                # emission so the (in-order) PE works through them in the
                # bubbles while it waits for later chunks' bf — instead of
                # serializing 20 matmuls after mmT3. DVE/ACT queue order is
                # unchanged (af0..3 then rden/corrclamp later).
                h0, h1 = ait
                dens, d2s = [], []
                emit_mmT(0)
                emit_mmT(1)
                emit_af(0)
                emit_mmT(2)
                emit_af(1)
                self_den(dens, d2s, 0, h0, h1, af[0])
                emit_mmT(3)
                emit_af(2)
                self_den(dens, d2s, 1, h0, h1, af[1])
                emit_af(3)
                self_den(dens, d2s, 2, h0, h1, af[2])
                self_den(dens, d2s, 3, h0, h1, af[3])
                return dens, d2s

            def self_den(dens, d2s, c, h0, h1, h2c):
                """Emit den = h2-2*h1+h0 (PE identity matmuls) and
                d2 = h2-h1 (Pool subtract, fp16 SBUF) for chunk c.

                d2 on the otherwise-idle Pool engine halves the PE load of
                the transition AND leaves the sA PSUM tags free, so the next
                block's w1*h2 matmul halves overlap the whole corr chain.
                d2 is Sterbenz-exact in fp16 for converging elements; the
                stray rounding on unconverged ones is damped by the clamp.
                """
                # den' = bias + k*(h2 - 2*h1 + h0): the bias row has no data
                # deps so it accumulates first; idf/nidf/n2idf hold +-kI
                denp = ppB.tile([128, FD], FP32, tag=f"sT{c}", name=f"denp{c}")
                nc.tensor.matmul(out=denp[:], lhsT=one_l[:], rhs=tiny_r[:],
                                 start=True, stop=False)
                nc.tensor.matmul(out=denp[:], lhsT=idf, rhs=h2c[:],
                                 start=False, stop=False)
                nc.tensor.matmul(out=denp[:], lhsT=n2idf, rhs=h1[c][:],
                                 start=False, stop=False)
                if isinstance(h0[c], tuple):
                    # block input was an extrapolation: h0 = h2p - corrp
                    h2p, corrp = h0[c]
                    nc.tensor.matmul(out=denp[:], lhsT=idf, rhs=h2p[:],
                                     start=False, stop=False)
                    nc.tensor.matmul(out=denp[:], lhsT=nidf, rhs=corrp[:],
                                     start=False, stop=True)
                else:
                    nc.tensor.matmul(out=denp[:], lhsT=idf, rhs=h0[c][:],
                                     start=False, stop=True)
                dens.append(denp)
                d2 = wpool.tile([128, FD], FP16, tag=f"d2{c}")
                nc.gpsimd.tensor_tensor(d2[:], h2c[:], h1[c][:],
                                        op=mybir.AluOpType.subtract)
                d2s.append(d2)

            def aitken_rest(dens, d2s, h2):
                """Fused smooth correction, one DVE op per chunk; af' =
                h2 - corr is never materialized — consumers fold the
                subtraction into their matmuls via -w1 / -m2 stationaries.
                """
                corrs = []
                for c in range(N_CHUNK):
                    corr = wpool.tile([128, FD], FP16, tag=f"corr{c}")
                    corrsmooth(nc.vector, corr[:], d2s[c][:], dens[c][:])
                    corrs.append(corr)
                for c in range(N_CHUNK):
                    af[c] = (h2[c], corrs[c])

            # ---- Steffensen blocks: n iterations each + Aitken ----
            # 2-iteration blocks use the block input itself as h0.
            for nb in [b for b in (n1, n2, n3) if b]:
                hist = [[atts[c] if af[c] is None else af[c]
                         for c in range(N_CHUNK)]]
                dd = None
                for k in range(nb):
                    last = do_aitken and k == nb - 1
                    dd = iteration(ait=(hist[-2], hist[-1]) if last else None)
                    hist.append(list(af))
                if do_aitken:
                    aitken_rest(*dd, h2=hist[-1])

            # ---- readout: bf* = 1/(1+S*), y = ones^T (m2^T af* . bf*) ----
            if not do_readout:
                continue
            # start the h2-halves of S* and g as soon as h2 exists (the
            # -corr accumulation joins once aitken's corr is ready)
            sA, gs, hs = [], [], []
            yts, gts = [], []
            for c in range(N_CHUNK):
                ps_t = ppA.tile([128, FD], FP32, tag=f"sA{c}", name=f"roA{c}")
                g_t = ppB.tile([128, FD], FP32, tag=f"sT{c}", name=f"roB{c}")
                yts.append(ps_t)
                gts.append(g_t)
                ps, g = ps_t[:], g_t[:]
                h2t, corrt = af[c]
                nc.tensor.matmul(out=ps, lhsT=w1f, rhs=h2t[:],
                                 start=True, stop=False)
                nc.tensor.matmul(out=g, lhsT=m2f, rhs=h2t[:],
                                 start=True, stop=False)
                sA.append(ps)
                gs.append(g)
            for c in range(N_CHUNK):
                h2t, corrt = af[c]
                stop_s = ro_bf_eng[c] != "pool"
                nc.tensor.matmul(out=sA[c], lhsT=nw1f, rhs=corrt[:],
                                 start=False, stop=stop_s)
                if not stop_s:   # pool bf needs the +1 materialized
                    nc.tensor.matmul(out=sA[c], lhsT=one_l[:], rhs=one_r[:],
                                     start=False, stop=True)
                nc.tensor.matmul(out=gs[c], lhsT=nm2f, rhs=corrt[:],
                                 start=False, stop=True)
            # bf* = 1/(1+S*): ACT (bias folds +1) or Pool exact divide.
            # h = g * bf on DVE (custom ops may read only ONE psum operand)
            # or Pool multiply.
            bfs = []
            for c in range(N_CHUNK):
                bf = wpool.tile([128, FD], FP16, tag=f"bf{c}")
                if ro_bf_eng[c] == "act":
                    _act_recip(nc, bf[:], sA[c])
                else:
                    # seedless 1/(1+S) on DVE: num=ones through DIV1P2
                    div1p2(nc.vector, bf[:], sA[c], ones_t[:])
                bfs.append(bf)
            # single merged y accumulator (8, FD) on chunk-0's ppA tile:
            # rows 0:4 = stream A chunk sums, rows 4:8 = stream B
            yp = yts[0]
            for c in range(N_CHUNK):
                h = wpool.tile([128, FD], FP16, tag=f"h{c}")
                if ro_h_eng[c] == "pool":
                    nc.gpsimd.tensor_mul(h[:], gs[c], bfs[c][:])
                else:
                    nc.vector.tensor_mul(h[:], gs[c], bfs[c][:])
                hs.append(h)
            for c in range(N_CHUNK):
                nc.tensor.matmul(out=yp[0:8, :],
                                 lhsT=ymask[:, 8 * c:8 * c + 8],
                                 rhs=hs[c][0:128, :],
                                 start=(c == 0), stop=(c == N_CHUNK - 1))
            if tail_mode == "scatter":
                nc.vector.tensor_copy(ys3[0:8, 0, :], yp[0:8, :])
                nc.gpsimd.trigger_dma(count=None)
                nc.gpsimd.wait_ge(ydma_sem, 16)
            else:
                ys = wpool.tile([128, FD], FP32, tag="ysA")
                nc.vector.tensor_copy(ys[0:8, :], yp[0:8, :])
                nc.sync.dma_start(out=yout[0:8, :], in_=ys[0:8, :])

    nc.finalize()
    return nc


N3 = None


def _get_module(repeat=1):
    key = f"nc{repeat}-{N1}-{N2}-{N3}"
    if key not in _CACHE:
        _CACHE[key] = _build_module(repeat, n1=N1, n2=N2, n3=N3)
    return _CACHE[key]


def _block_diag(m):
    out = np.zeros((128, 128), np.float16)
    out[:64, :64] = m
    out[64:, 64:] = m
    return out


def kernel(AT, K_raw, BT_raw, W_raw, b_raw, _run_kw=None, _repeat=1):
    AT = np.asarray(AT, dtype=np.float32)
    K = np.clip(np.exp(np.asarray(K_raw, np.float32)), 0.0, 1000.0).astype(np.float32)
    BT = np.clip(np.exp(np.asarray(BT_raw, np.float32)), 0.0, 1000.0).astype(np.float32)
    Wc = np.clip(np.asarray(W_raw, np.float32), -10.0, 10.0).reshape(NA, NB)
    b0 = np.clip(np.asarray(b_raw, np.float32), -10.0, 10.0)[0]

    w1 = _block_diag(K.astype(np.float16))                       # S = K^T af
    w2 = _block_diag((K * BT[None, :]).T.astype(np.float16))     # T = w2^T bf
    m2 = _block_diag((K * Wc * BT[None, :]).astype(np.float16))  # bilinear
    # aitken identity stationaries are pre-scaled by k (den' = k*den)
    kident = np.float16(AIT_K) * np.eye(128, dtype=np.float16)
    pack2 = np.ascontiguousarray(
        np.concatenate([m2, kident, -kident, -2 * kident, -w1, -m2], axis=1))

    att = np.ascontiguousarray(AT.T.astype(np.float16))          # (64, 16384)

    # scatter-add row indices: row i from SBUF partition i (16 rows; the
    # last 8 are padding rows of yout the host ignores)
    idxs = np.arange(16, dtype=np.int16).reshape(16, 1)

    in_maps = []
    for c in range(N_CORES):
        chunk = att[:, c * B_CORE:(c + 1) * B_CORE]              # (64, 2048)
        stacked = np.concatenate([chunk[:, :B_CORE // 2], chunk[:, B_CORE // 2:]],
                                 axis=0)                         # (128, 1024)
        packA = np.ascontiguousarray(
            np.concatenate([w1, w2, stacked[:, :2 * FD]], axis=1))
        packB = np.ascontiguousarray(stacked[:, 2 * FD:])
        in_maps.append({"packA": packA, "packB": packB, "pack2": pack2,
                        "idxs": idxs})

    nc = _get_module(_repeat)
    res = run_bass_kernel_spmd(nc, in_maps, core_ids=list(range(N_CORES)),
                               **(_run_kw or {}))
    out = np.empty((B,), np.float32)
    half = B_CORE // 2
    for c in range(N_CORES):
        yo = res.results[c]["yout"][0:8].reshape(2, N_CHUNK, FD)
        base = c * B_CORE
        for ch in range(N_CHUNK):
            out[base + ch * FD:base + (ch + 1) * FD] = yo[0, ch]
            out[base + half + ch * FD:base + half + (ch + 1) * FD] = yo[1, ch]
    if _run_kw is not None:
        _CACHE["last_result"] = res
    return out + b0



# revision 60
# speedup vs baseline: 1.0433x; 1.0433x over previous
"""Trainium2 Bass kernel for nn_CompetitiveNetwork (competitive-binding solve).

Final design, ~3.9x faster than the 21-plain-iteration baseline:
  - Steffensen acceleration: two blocks of (4, 3) fixed-point iterations,
    each followed by a per-element Aitken extrapolation. 7 matmul-pair
    iterations replace the reference's 21 (contraction ~0.6/iter).
  - All reciprocals are SEEDLESS: bf on ACT (Reciprocal, +1 via bias);
    af = AT/(1+T) in ONE custom DVE op (DIV1P: bitwise-NOT seed + one
    Newton pass + multiply), so no Pool muls and no Newton-seed hazards.
  - Aitken: d2/den built on the idle PE via +/-identity matmuls into
    PSUM; corr = d2*clip(d2*rden, +-4) (CORRCLAMP custom op) guards the
    fp16 quantization blow-up; af' = h2-corr is never materialized --
    consumers fold the subtraction via -w1/-m2 stationaries.
  - 2 packed input DMAs, parallel per-stream output copy+DMA tail.


Math per batch row b (host precomputes K = clip(exp(K_raw)), BT = 1,
w1 = K, w2 = (K*BT)^T, m2 = K*clip(W)*BT):
    iterate:  bf = 1/(1 + K^T af);  af = AT * 1/(1 + w2^T bf)
    readout:  y  = sum_j (m2^T af)_j * bf_j + b

Acceleration: Steffensen — two blocks of (4, 3) fixed-point iterations,
each followed by a per-element Aitken extrapolation (x2 - d2^2/(d2-d1)).
7 iterations + 2 extrapolations match the 21-iteration reference to
~2-3e-3 relative (fixed-point contraction ~0.6/iter; validated vs the
fp32 reference in numpy with exact fp16/fp32 device arithmetic).

Device layout: data-parallel over batch (16384 -> 8 cores x 2048);
transposed state (64 features on partitions, batch on free), two
64-partition streams stacked to (128, FD) tiles, 4 chunks of FD=256.
Matmuls use 128x128 block-diagonal stationaries (one instruction per
chunk instead of two 64x64 quadrant matmuls).

Engine plan per iteration (layered emission for cross-chunk overlap):
    PE : mmS[c] (block-diag w1, fp16) ; bias row + mmT[c] accumulating
         1 + w2^T bf into PSUM
    ACT: bf[c] = Reciprocal(S[c] + 1) -> fp16   (bias folds the +1)
    DVE: r[c] = reciprocal_approx_fast(1+T[c]) -> fp32 (seedless, ~18 bits)
    Pool: af[c] = att16[c] * r[c] -> fp16
All reciprocals are seedless/exact: no Newton-seed divergence hazards.
"""

import numpy as np

import concourse.bacc as bacc
import concourse.mybir as mybir
from concourse.tile import TileContext
from concourse.bass_utils import run_bass_kernel_spmd

# --- custom DVE op CORRCLAMP_ANT: out = Src0 * clip(Src0*Src1, C0, C1) ---
# The Aitken correction d2^2/den with the ratio d2*(1/den) clamped to the
# range valid for geometric sequences (rate <= 0.65 => |d2/den| <= ~3).
# Clamping kills the blow-up when consecutive fp16 differences quantize
# equal (den ~ 0 while d2 is a 1-2 ulp residue).

import concourse.dve_ops as dve_ops
from concourse.dve_ops import DveOp
from concourse.dve_spec import (Spec, Src0, Src1, C0, C1, C2, lower, maxx,
                                minn, Bin, AluOp)


def _ref_corrclamp(in0, in1, c0, c1, c2):
    q = np.clip(in0.astype(np.float32) * in1.astype(np.float32), c0, c1)
    return (q * in0.astype(np.float32)).astype(np.float32)


_not_x = Bin(AluOp.BITWISE_NOT, Src0, Src0)
_y0 = _not_x * C0
_y1 = _y0 * (C1 - Src0 * _y0)

# bit-flip seed + one NR pass + multiply: out = Src1 * ~1/Src0, rel err
# ~1.7e-3 (one Newton pass short of RECIPROCAL_APPROX_FAST). Good enough
# inside the fixed-point loop where the contraction damps recip noise.
def _ref_div1p(in0, in1, c0, c1, c2):
    x = in0.astype(np.float32)
    not_x = (~x.view(np.int32)).view(np.float32)
    y0 = not_x * np.float32(c0)
    y1 = y0 * (np.float32(c1) - x * y0)
    return (y1 * in1.astype(np.float32)).astype(np.float32)


# DIV1P2: same seedless divide but with the +1 folded into the op
# (x1 = Src0 + 1), so the PE bias-row matmul into sT is unnecessary.
from concourse.dve_spec import One as _One

_x1 = Src0 + _One
_not_x1 = Bin(AluOp.BITWISE_NOT, _x1, _x1)
_y0b = _not_x1 * C0
_y1b = _y0b * (C1 - _x1 * _y0b)


def _ref_div1p2(in0, in1, c0, c1, c2):
    x = np.ascontiguousarray(in0.astype(np.float32) + np.float32(1.0))
    not_x = (~x.view(np.int32)).view(np.float32)
    y0 = not_x * np.float32(c0)
    y1 = y0 * (np.float32(c1) - x * y0)
    return (y1 * in1.astype(np.float32)).astype(np.float32)


# CORRSMOOTH: corr = k*d2^2*den' / (d2^2 + den'^2), where the CALLER
# supplies den' = k*den + bias accumulated on the PE (scaled +-kI identity
# stationaries plus a 2^-12 bias row). This is the smooth equivalent of
# clip(d2/den, +-k/2)*d2 with graceful 0/0 -> 0 handling: no clamp, no NaN
# path, no ACT rden at all. The DVE pipeline budget is 8 ALU ops, so the
# reciprocal is SEED-ONLY (bitwise-NOT, rel err ~6%, benign on a damped
# extrapolation correction) and the k fold rides inside the seed constant:
# ops = [d2^2, den'^2, +, NOT, *C2, d2*den', *d2, *y0] = exactly 8.
# Src0 = d2, Src1 = den'; C2 = k * seed_scale (C0/C1 unused).

_x = Src0 * Src0 + Src1 * Src1
_nx = Bin(AluOp.BITWISE_NOT, _x, _x)
_ry0 = _nx * C2


def _ref_corrsmooth(in0, in1, c0, c1, c2):
    d2 = in0.astype(np.float32)
    den = in1.astype(np.float32)
    x = np.ascontiguousarray(d2 * d2 + den * den)
    not_x = (~x.view(np.int32)).view(np.float32)
    y0 = not_x * np.float32(c2)
    return (((d2 * den) * d2) * y0).astype(np.float32)


_SPECS = {
    "CORRCLAMP_ANT": Spec(body=minn(maxx(Src0 * Src1, C0), C1) * Src0,
                          reference=_ref_corrclamp),
    "DIV1P_ANT": Spec(body=_y1 * Src1, reference=_ref_div1p),
    "DIV1P2_ANT": Spec(body=_y1b * Src1, reference=_ref_div1p2),
    "CORRSMOOTH_ANT": Spec(body=((Src0 * Src1) * Src0) * _ry0,
                           reference=_ref_corrsmooth),
}


def _make_op(name, shas):
    return DveOp(name, _SPECS[name], subdim=False, uops_sha=shas)


def _register(name):
    for op in dve_ops.OPS:
        if op.name == name:
            return op
    probe = _make_op(name, {})
    shas = {}
    for ver in ("v3", "v4"):
        try:
            from concourse.dve_uop import DveOpSpec
            opcode = dve_ops._CUSTOM_DVE_ROW_BASE + len(dve_ops.OPS)
            res = DveOpSpec(name=probe.name, opcode=opcode,
                            uops=lower(probe.spec, ver=ver),
                            rd1_en=True)
            shas[ver] = res.sha(ver)
        except Exception as e:
            print(f"lower {name} {ver} failed: {e}")
    op = _make_op(name, shas)
    dve_ops.OPS.append(op)
    dve_ops.CUSTOM_DVE_SPECS[op.name] = op.spec
    dve_ops._SUB_OPCODE_FOR_NAME[op.name] = (
        dve_ops._CUSTOM_DVE_ROW_BASE + len(dve_ops.OPS) - 1)
    return op


def corrclamp(nc_vector, out, d2, rden, lim=4.0):
    op = _register("CORRCLAMP_ANT")
    return nc_vector._custom_dve(op, out=out, in0=d2, in1=rden,
                                 s0=-lim, s1=lim, imm2=0.0)


def div1p(nc_vector, out, x, num):
    op = _register("DIV1P_ANT")
    return nc_vector._custom_dve(op, out=out, in0=x, in1=num,
                                 s0=-0.23549792, s1=2.0017324, imm2=0.0)


def div1p2(nc_vector, out, x, num):
    """out = num / (1 + x), seedless (folds the +1; no bias row needed)."""
    op = _register("DIV1P2_ANT")
    return nc_vector._custom_dve(op, out=out, in0=x, in1=num,
                                 s0=-0.23549792, s1=2.0017324, imm2=0.0)


AIT_K = 32.0                   # den scale; cap |d2/den| at k/2
AIT_BIAS = 2.0 ** -12          # den' bias row value (zero-protection)
_SEED_A = -0.23528764          # seed-only NOT-reciprocal scale


def corrsmooth(nc_vector, out, d2, den):
    op = _register("CORRSMOOTH_ANT")
    return nc_vector._custom_dve(op, out=out, in0=d2, in1=den,
                                 s0=0.0, s1=0.0, imm2=AIT_K * _SEED_A)

B, NA, NB = 16384, 64, 64
N_CORES = 8
B_CORE = B // N_CORES          # 2048 batch rows per core
N_CHUNK = 4
FD = B_CORE // 2 // N_CHUNK    # 256
N1, N2 = 3, 3                  # Steffensen blocks

FP32 = mybir.dt.float32
FP16 = mybir.dt.float16

_CACHE = {}


def _act_recip(nc, out_ap, in_ap, bias=1.0):
    eng = nc.scalar
    ins = [eng.lower_ap(in_ap),
           mybir.ImmediateValue(dtype=FP32, value=bias),   # bias: +1
           mybir.ImmediateValue(dtype=FP32, value=1.0),    # scale
           mybir.ImmediateValue(dtype=FP32, value=0.0)]
    eng.add_instruction(mybir.InstActivation(
        name=nc.get_next_instruction_name(),
        func=mybir.ActivationFunctionType.Reciprocal,
        ins=ins, outs=[eng.lower_ap(out_ap)]))


def _build_module(repeat=1, n1=N1, n2=N2, n3=None, do_aitken=True,
                  do_readout=True,
                  mul_eng="pool", div1p_mode="all", ait_eng="pe",
                  bf_eng=("act", "act", "act", "act"),
                  af_eng=("dve", "dve", "dve", "dve"),
                  ro_bf_eng=("act", "act", "act", "act"),
                  ro_h_eng=("dve", "dve", "dve", "dve"),
                  tail_mode="copy"):
    nc = bacc.Bacc()
    # packA: w1 | w2 | att0  (first-iteration critical data: arrives first)
    # packB: att1 | att2 | att3
    # pack2: m2 | I | -I | -2I | -w1 | -m2 (readout + aitken consts)
    packA = nc.dram_tensor("packA", (128, 256 + 2 * FD), FP16,
                           kind="ExternalInput")
    packB = nc.dram_tensor("packB", (128, (N_CHUNK - 2) * FD), FP16,
                           kind="ExternalInput")
    pack2 = nc.dram_tensor("pack2", (128, 768), FP16, kind="ExternalInput")
    # rows 0:8 carry the result; rows 8:16 are scatter-mode padding (the
    # SWDGE scatter needs 16 non-negative indices to write rows 0-7 cleanly)
    yout = nc.dram_tensor("yout", (16, FD), FP32, kind="ExternalOutput")
    if tail_mode == "scatter":
        idxs_d = nc.dram_tensor("idxs", (16, 1), mybir.dt.int16,
                                kind="ExternalInput")

    with TileContext(nc) as tc, \
         tc.tile_pool(name="const", bufs=1) as cpool, \
         tc.tile_pool(name="state", bufs=5) as spool, \
         tc.tile_pool(name="work", bufs=8) as wpool, \
         tc.tile_pool(name="psA", bufs=1, space="PSUM") as ppA, \
         tc.tile_pool(name="psB", bufs=1, space="PSUM") as ppB:

        p1a = cpool.tile([128, 256 + 2 * FD], FP16, tag="p1a")
        p1b = cpool.tile([128, (N_CHUNK - 2) * FD], FP16, tag="p1b")
        p2 = cpool.tile([128, 768], FP16, tag="p2")
        nc.sync.dma_start(out=p1a[:], in_=packA[:, :])
        nc.sync.dma_start(out=p1b[:], in_=packB[:, :])
        nc.sync.dma_start(out=p2[:], in_=pack2[:, :])
        w1f = p1a[:, 0:128]
        w2f = p1a[:, 128:256]
        atts = [p1a[:, 256 + c * FD:256 + (c + 1) * FD] for c in range(2)] + \
               [p1b[:, c * FD:(c + 1) * FD] for c in range(N_CHUNK - 2)]
        m2f = p2[:, 0:128]
        idf = p2[:, 128:256]      # I
        nidf = p2[:, 256:384]     # -I
        n2idf = p2[:, 384:512]    # -2I
        nw1f = p2[:, 512:640]     # -w1
        nm2f = p2[:, 640:768]     # -m2
        one_l = cpool.tile([1, 128], FP16, tag="one_l")   # bias lhsT (1,128)
        nc.vector.memset(one_l[:], 1.0)
        one_r = cpool.tile([1, FD], FP16, tag="one_r")    # bias rhs (1,FD)
        nc.vector.memset(one_r[:], 1.0)

        ones_t = cpool.tile([128, FD], FP16, tag="ones_t")  # Pool bf numerator
        nc.vector.memset(ones_t[:], 1.0)
        # readout reduction mask (merged streams): chunk c uses columns
        # [8c:8c+8]; col 8c+j has ones on partitions 0:64 iff j==c (stream A
        # sums land in out rows 0:4), col 8c+4+j ones on partitions 64:128
        # iff j==c (stream B sums land in out rows 4:8).
        ymask = cpool.tile([128, 8 * N_CHUNK], FP16, tag="ymask")
        nc.vector.memset(ymask[:], 0.0)
        for c in range(N_CHUNK):
            nc.vector.memset(ymask[0:64, 8 * c + c:8 * c + c + 1], 1.0)
            nc.vector.memset(ymask[64:128, 8 * c + 4 + c:8 * c + 4 + c + 1], 1.0)

        if tail_mode == "scatter":
            # y leaves via a pre-generated SWDGE scatter-add: descriptors are
            # built off the critical path, so the tail is just copy ->
            # trigger -> tiny transfer instead of a full dma_start chain
            # (config + HWDGE issuance + DGE delay). yout is zeroed early by
            # an overlapped DMA since the scatter ADDs into it.
            from concourse import library_config
            nc.gpsimd.load_library(library_config.mlp)
            idxs_t = cpool.tile([16, 1], mybir.dt.int16, tag="idxs")
            nc.sync.dma_start(out=idxs_t[:], in_=idxs_d[:, :])
            zeros_t = cpool.tile([128, FD], FP32, tag="zeros")
            nc.vector.memset(zeros_t[:], 0.0)
            nc.sync.dma_start(out=yout[:, :], in_=zeros_t[0:16, :])
            ys3 = cpool.tile([128, 1, FD], FP32, tag="ys3")
            nc.vector.memset(ys3[:, :, :], 0.0)
            ydma_sem = nc.alloc_semaphore("ydma")
            nc.gpsimd.dma_scatter_add(yout[:, :], ys3[:, :, :], idxs_t[:],
                                      16, 16, FD, prepare_only=True,
                                      sem=ydma_sem)
        else:
            ys3 = ydma_sem = None

        for _rep in range(repeat):
            af = [None] * N_CHUNK

            def iteration(ait=None):
                """One fixed-point iteration over all chunks, layered.

                Per-chunk engine lanes (bf_eng / af_eng): "act" = ACT
                Reciprocal (bias folds the +1), "dve" = DIV1P2 custom op
                (folds the +1), "pool" = GPSIMD exact divide (needs an
                explicit +1 bias row accumulated into its PSUM tile).
                """
                sA, sT, bfs = [], [], []
                # two passes for extrapolated inputs: the w1*h2 halves have
                # no corr dependency, so emit them all first — the PE works
                # through them while the corr chain (den->rden->corrclamp)
                # is still in flight, instead of head-of-line blocking.
                for c in range(N_CHUNK):
                    ps_t = ppA.tile([128, FD], FP32, tag=f"sA{c}", name=f"sA{c}")
                    ps = ps_t[:]
                    need_bias = bf_eng[c] == "pool"
                    if af[c] is None:
                        nc.tensor.matmul(out=ps, lhsT=w1f, rhs=atts[c],
                                         start=True, stop=not need_bias)
                    elif isinstance(af[c], tuple):
                        nc.tensor.matmul(out=ps, lhsT=w1f, rhs=af[c][0][:],
                                         start=True, stop=False)
                    else:
                        nc.tensor.matmul(out=ps, lhsT=w1f, rhs=af[c][:],
                                         start=True, stop=not need_bias)
                    sA.append(ps)
                for c in range(N_CHUNK):
                    need_bias = bf_eng[c] == "pool"
                    if isinstance(af[c], tuple):
                        nc.tensor.matmul(out=sA[c], lhsT=nw1f, rhs=af[c][1][:],
                                         start=False, stop=not need_bias)
                    if need_bias:
                        nc.tensor.matmul(out=sA[c], lhsT=one_l[:], rhs=one_r[:],
                                         start=False, stop=True)
                # sT bias rows only for pool-af chunks (ACT folds via bias,
                # DVE folds via DIV1P2)
                for c in range(N_CHUNK):
                    ps2_t = ppB.tile([128, FD], FP32, tag=f"sT{c}", name=f"sT{c}")
                    sT.append(ps2_t[:])
                    if af_eng[c] == "pool":
                        nc.tensor.matmul(out=sT[c], lhsT=one_l[:], rhs=one_r[:],
                                         start=True, stop=False)
                def emit_bf(c):
                    bf = wpool.tile([128, FD], FP16, tag=f"bf{c}")
                    if bf_eng[c] == "act":
                        _act_recip(nc, bf[:], sA[c])
                    else:
                        nc.gpsimd.tensor_tensor(bf[:], ones_t[:], sA[c],
                                                op=mybir.AluOpType.divide)
                    bfs.append(bf)

                def emit_mmT(c):
                    first = af_eng[c] != "pool"
                    nc.tensor.matmul(out=sT[c], lhsT=w2f, rhs=bfs[c][:],
                                     start=first, stop=True)

                def emit_af(c):
                    af_n = spool.tile([128, FD], FP16, tag=f"af{c}")
                    if af_eng[c] == "dve":
                        div1p2(nc.vector, af_n[:], sT[c], atts[c])
                    else:
                        nc.gpsimd.tensor_tensor(af_n[:], atts[c], sT[c],
                                                op=mybir.AluOpType.divide)
                    af[c] = af_n

                for c in range(N_CHUNK):
                    emit_bf(c)
                if ait is None:
                    for c in range(N_CHUNK):
                        emit_mmT(c)
                    for c in range(N_CHUNK):
                        emit_af(c)
                    return None
                # Last iteration of a Steffensen block: interleave the
                # aitken den/d2 identity-matmuls with this iteration's mmT
                # emission so the (in-order) PE works through them in the
                # bubbles while it waits for later chunks' bf — instead of
                # serializing 20 matmuls after mmT3. DVE/ACT queue order is
                # unchanged (af0..3 then rden/corrclamp later).
                h0, h1 = ait
                dens, d2s = [], []
                emit_mmT(0)
                emit_mmT(1)
                emit_af(0)
                emit_mmT(2)
                emit_af(1)
                self_den(dens, d2s, 0, h0, h1, af[0])
                emit_mmT(3)
                emit_af(2)
                self_den(dens, d2s, 1, h0, h1, af[1])
                emit_af(3)
                self_den(dens, d2s, 2, h0, h1, af[2])
                self_den(dens, d2s, 3, h0, h1, af[3])
                return dens, d2s

            def self_den(dens, d2s, c, h0, h1, h2c):
                """Emit den = h2-2*h1+h0 (PE identity matmuls) and
                d2 = h2-h1 (Pool subtract, fp16 SBUF) for chunk c.

                d2 on the otherwise-idle Pool engine halves the PE load of
                the transition AND leaves the sA PSUM tags free, so the next
                block's w1*h2 matmul halves overlap the whole corr chain.
                d2 is Sterbenz-exact in fp16 for converging elements; the
                stray rounding on unconverged ones is damped by the clamp.
                """
                denp = ppB.tile([128, FD], FP32, tag=f"sT{c}", name=f"denp{c}")
                nc.tensor.matmul(out=denp[:], lhsT=idf, rhs=h2c[:],
                                 start=True, stop=False)
                nc.tensor.matmul(out=denp[:], lhsT=n2idf, rhs=h1[c][:],
                                 start=False, stop=False)
                if isinstance(h0[c], tuple):
                    # block input was an extrapolation: h0 = h2p - corrp
                    h2p, corrp = h0[c]
                    nc.tensor.matmul(out=denp[:], lhsT=idf, rhs=h2p[:],
                                     start=False, stop=False)
                    nc.tensor.matmul(out=denp[:], lhsT=nidf, rhs=corrp[:],
                                     start=False, stop=True)
                else:
                    nc.tensor.matmul(out=denp[:], lhsT=idf, rhs=h0[c][:],
                                     start=False, stop=True)
                dens.append(denp)
                d2 = wpool.tile([128, FD], FP16, tag=f"d2{c}")
                nc.gpsimd.tensor_tensor(d2[:], h2c[:], h1[c][:],
                                        op=mybir.AluOpType.subtract)
                d2s.append(d2)

            def aitken_rest(dens, d2s, h2):
                """rden (ACT) + clamped correction (DVE), parallel lanes;
                af' = h2 - corr is never materialized — consumers fold the
                subtraction into their matmuls via -w1 / -m2 stationaries.
                The rden 1e-12 bias keeps corr = 0 (not NaN) for fully
                converged elements where d2 = den = 0.
                """
                rdens, corrs = [], []
                for c in range(N_CHUNK):
                    rden = wpool.tile([128, FD], FP32, tag=f"rden{c}")
                    _act_recip(nc, rden[:], dens[c][:], bias=1e-12)
                    rdens.append(rden)
                for c in range(N_CHUNK):
                    corr = wpool.tile([128, FD], FP16, tag=f"corr{c}")
                    corrclamp(nc.vector, corr[:], d2s[c][:], rdens[c][:])
                    corrs.append(corr)
                for c in range(N_CHUNK):
                    af[c] = (h2[c], corrs[c])

            # ---- Steffensen blocks: n iterations each + Aitken ----
            # 2-iteration blocks use the block input itself as h0.
            for nb in [b for b in (n1, n2, n3) if b]:
                hist = [[atts[c] if af[c] is None else af[c]
                         for c in range(N_CHUNK)]]
                dd = None
                for k in range(nb):
                    last = do_aitken and k == nb - 1
                    dd = iteration(ait=(hist[-2], hist[-1]) if last else None)
                    hist.append(list(af))
                if do_aitken:
                    aitken_rest(*dd, h2=hist[-1])

            # ---- readout: bf* = 1/(1+S*), y = ones^T (m2^T af* . bf*) ----
            if not do_readout:
                continue
            # start the h2-halves of S* and g as soon as h2 exists (the
            # -corr accumulation joins once aitken's corr is ready)
            sA, gs, hs = [], [], []
            yts, gts = [], []
            for c in range(N_CHUNK):
                ps_t = ppA.tile([128, FD], FP32, tag=f"sA{c}", name=f"roA{c}")
                g_t = ppB.tile([128, FD], FP32, tag=f"sT{c}", name=f"roB{c}")
                yts.append(ps_t)
                gts.append(g_t)
                ps, g = ps_t[:], g_t[:]
                h2t, corrt = af[c]
                nc.tensor.matmul(out=ps, lhsT=w1f, rhs=h2t[:],
                                 start=True, stop=False)
                nc.tensor.matmul(out=g, lhsT=m2f, rhs=h2t[:],
                                 start=True, stop=False)
                sA.append(ps)
                gs.append(g)
            for c in range(N_CHUNK):
                h2t, corrt = af[c]
                stop_s = ro_bf_eng[c] != "pool"
                nc.tensor.matmul(out=sA[c], lhsT=nw1f, rhs=corrt[:],
                                 start=False, stop=stop_s)
                if not stop_s:   # pool bf needs the +1 materialized
                    nc.tensor.matmul(out=sA[c], lhsT=one_l[:], rhs=one_r[:],
                                     start=False, stop=True)
                nc.tensor.matmul(out=gs[c], lhsT=nm2f, rhs=corrt[:],
                                 start=False, stop=True)
            # bf* = 1/(1+S*): ACT (bias folds +1) or Pool exact divide.
            # h = g * bf on DVE (custom ops may read only ONE psum operand)
            # or Pool multiply.
            bfs = []
            for c in range(N_CHUNK):
                bf = wpool.tile([128, FD], FP16, tag=f"bf{c}")
                if ro_bf_eng[c] == "act":
                    _act_recip(nc, bf[:], sA[c])
                else:
                    # seedless 1/(1+S) on DVE: num=ones through DIV1P2
                    div1p2(nc.vector, bf[:], sA[c], ones_t[:])
                bfs.append(bf)
            # single merged y accumulator (8, FD) on chunk-0's ppA tile:
            # rows 0:4 = stream A chunk sums, rows 4:8 = stream B
            yp = yts[0]
            for c in range(N_CHUNK):
                h = wpool.tile([128, FD], FP16, tag=f"h{c}")
                if ro_h_eng[c] == "pool":
                    nc.gpsimd.tensor_mul(h[:], gs[c], bfs[c][:])
                else:
                    nc.vector.tensor_mul(h[:], gs[c], bfs[c][:])
                hs.append(h)
            for c in range(N_CHUNK):
                nc.tensor.matmul(out=yp[0:8, :],
                                 lhsT=ymask[:, 8 * c:8 * c + 8],
                                 rhs=hs[c][0:128, :],
                                 start=(c == 0), stop=(c == N_CHUNK - 1))
            if tail_mode == "scatter":
                nc.vector.tensor_copy(ys3[0:8, 0, :], yp[0:8, :])
                nc.gpsimd.trigger_dma(count=None)
                nc.gpsimd.wait_ge(ydma_sem, 16)
            else:
                ys = wpool.tile([128, FD], FP32, tag="ysA")
                nc.vector.tensor_copy(ys[0:8, :], yp[0:8, :])
                nc.sync.dma_start(out=yout[0:8, :], in_=ys[0:8, :])

    nc.finalize()
    return nc


N3 = None


def _get_module(repeat=1):
    key = f"nc{repeat}-{N1}-{N2}-{N3}"
    if key not in _CACHE:
        _CACHE[key] = _build_module(repeat, n1=N1, n2=N2, n3=N3)
    return _CACHE[key]


def _block_diag(m):
    out = np.zeros((128, 128), np.float16)
    out[:64, :64] = m
    out[64:, 64:] = m
    return out


def kernel(AT, K_raw, BT_raw, W_raw, b_raw, _run_kw=None, _repeat=1):
    AT = np.asarray(AT, dtype=np.float32)
    K = np.clip(np.exp(np.asarray(K_raw, np.float32)), 0.0, 1000.0).astype(np.float32)
    BT = np.clip(np.exp(np.asarray(BT_raw, np.float32)), 0.0, 1000.0).astype(np.float32)
    Wc = np.clip(np.asarray(W_raw, np.float32), -10.0, 10.0).reshape(NA, NB)
    b0 = np.clip(np.asarray(b_raw, np.float32), -10.0, 10.0)[0]

    w1 = _block_diag(K.astype(np.float16))                       # S = K^T af
    w2 = _block_diag((K * BT[None, :]).T.astype(np.float16))     # T = w2^T bf
    m2 = _block_diag((K * Wc * BT[None, :]).astype(np.float16))  # bilinear
    ident = np.eye(128, dtype=np.float16)
    pack2 = np.ascontiguousarray(
        np.concatenate([m2, ident, -ident, -2 * ident, -w1, -m2], axis=1))

    att = np.ascontiguousarray(AT.T.astype(np.float16))          # (64, 16384)

    # scatter-add row indices: row i from SBUF partition i (16 rows; the
    # last 8 are padding rows of yout the host ignores)
    idxs = np.arange(16, dtype=np.int16).reshape(16, 1)

    in_maps = []
    for c in range(N_CORES):
        chunk = att[:, c * B_CORE:(c + 1) * B_CORE]              # (64, 2048)
        stacked = np.concatenate([chunk[:, :B_CORE // 2], chunk[:, B_CORE // 2:]],
                                 axis=0)                         # (128, 1024)
        packA = np.ascontiguousarray(
            np.concatenate([w1, w2, stacked[:, :2 * FD]], axis=1))
        packB = np.ascontiguousarray(stacked[:, 2 * FD:])
        in_maps.append({"packA": packA, "packB": packB, "pack2": pack2,
                        "idxs": idxs})

    nc = _get_module(_repeat)
    res = run_bass_kernel_spmd(nc, in_maps, core_ids=list(range(N_CORES)),
                               **(_run_kw or {}))
    out = np.empty((B,), np.float32)
    half = B_CORE // 2
    for c in range(N_CORES):
        yo = res.results[c]["yout"][0:8].reshape(2, N_CHUNK, FD)
        base = c * B_CORE
        for ch in range(N_CHUNK):
            out[base + ch * FD:base + (ch + 1) * FD] = yo[0, ch]
            out[base + half + ch * FD:base + half + (ch + 1) * FD] = yo[1, ch]
    if _run_kw is not None:
        _CACHE["last_result"] = res
    return out + b0

